# revision 3
# baseline (speedup 1.0000x reference)
"""Causal multi-head attention on 8 TRN2 NeuronCores.

Sharding: core c -> (batch b = c // 2, head-half hh = c % 2).
Each core computes QKV for its 8 heads over the full sequence of its batch,
causal flash attention, and a partial out-projection using its 512 rows of
w_out. The host sums the two partials per batch (the "all-reduce" of the
tensor-parallel out projection).

v3 layout (all matmul operands bf16):
  xs[d]    [128, 2048] whole input row-block, loaded once
  KT[c][j] [128, 512]  K^T head pair j, token chunk c (head 2j rows 0:64,
                       head 2j+1 rows 64:128)
  V[t]     [128, 584]  V token-tile t, 8 heads x (64 cols + ones col),
                       padded so AV can load 128-col weight slices (FWL)
  QT[c][j] [128, 512]  Q^T (even head rows 0:64, odd rows 64:128)

Per k-block pair, scores for both heads land in ONE 4-bank PSUM tile
  s_all = [a-even | b-even | a-odd | b-odd]  (4 x 512 cols)
so a single wide EXP covers the pair and all four S matmuls become ready
together; emitted as (a-even,a-odd),(b-even,b-odd) they run pairwise
CONCURRENTLY on the PE via K=64 row tiling (tile_position (0,0)/(64,0)).

AV uses lhsT = V[pk][:, 65*h : 65*h+128] (128-col weight loads -> FWL);
output rows 65:127 are garbage that lands in unread PSUM partitions.

Schedule: QKV projection for chunk c+1 and the out-projection for chunk
c-1 are woven into the attention pair loop of chunk c.

Shapes (hardcoded): B=4, T=2048, D=1024, H=16, HD=64.
"""
import sys

for _p in ('/opt/trn_rl_repo', '/root/.axon_site/_ro/trn_rl_repo'):
    if _p not in sys.path:
        sys.path.insert(0, _p)

import numpy as np

B, T, D = 4, 2048, 1024
H, HD = 16, 64
HPC = H // 2          # heads per core = 8
DPC = HPC * HD        # out-dims per core = 512
N_CORES = 8

_nc_cache = {}


def _build_nc():
    import concourse.bacc as bacc
    import concourse.mybir as mybir
    from concourse.tile import TileContext

    F32 = mybir.dt.float32
    BF16 = mybir.dt.bfloat16
    AF = mybir.ActivationFunctionType
    ALU = mybir.AluOpType

    CH = 512              # token chunk (both proj and attention q-chunk)
    NKB = T // 128        # 16 k-blocks
    NC = T // CH          # 4 chunks
    NDT = D // 128        # 8 input-dim tiles
    VW = HPC * (HD + 1) + 64   # V tile width = 584 (64-col pad for LDW)

    nc = bacc.Bacc('TRN2', target_bir_lowering=False, debug=False)
    xT_d = nc.dram_tensor('xT', [D, T], BF16, kind='ExternalInput')
    wq_d = nc.dram_tensor('wq', [D, DPC], BF16, kind='ExternalInput')
    wk_d = nc.dram_tensor('wk', [D, DPC], BF16, kind='ExternalInput')
    wv_d = nc.dram_tensor('wv', [D, DPC], BF16, kind='ExternalInput')
    wo_d = nc.dram_tensor('wo', [DPC, D], BF16, kind='ExternalInput')
    po_d = nc.dram_tensor('po', [T, D], F32, kind='ExternalOutput')

    with nc.allow_low_precision(reason='bf16 matmuls by design'), \
            TileContext(nc) as tc:
        with (
            tc.tile_pool(name='w', bufs=1) as w_pool,
            tc.tile_pool(name='kt', bufs=1) as kt_pool,
            tc.tile_pool(name='vv', bufs=1) as v_pool,
            tc.tile_pool(name='xs', bufs=1) as x_pool,
            tc.tile_pool(name='qt', bufs=2) as qt_pool,
            tc.tile_pool(name='pt', bufs=3) as pt_pool,
            tc.tile_pool(name='ao', bufs=2) as ao_pool,
            tc.tile_pool(name='osb', bufs=2) as osb_pool,
            tc.tile_pool(name='small', bufs=2) as sm_pool,
            tc.tile_pool(name='ps_s', bufs=1, space='PSUM') as ps_s,
            tc.tile_pool(name='ps_ot', bufs=4, space='PSUM') as ps_ot,
        ):
            WK = [w_pool.tile([128, DPC], BF16, tag=f'wk{d}', name=f'wks{d}')
                  for d in range(NDT)]
            WV = [w_pool.tile([128, DPC], BF16, tag=f'wv{d}', name=f'wvs{d}')
                  for d in range(NDT)]
            WQ = [w_pool.tile([128, DPC], BF16, tag=f'wq{d}', name=f'wqs{d}')
                  for d in range(NDT)]
            WO = [w_pool.tile([128, D], BF16, tag=f'wo{d}', name=f'wos{d}')
                  for d in range(4)]
            XS = [x_pool.tile([128, T], BF16, tag=f'x{d}', name=f'xs{d}')
                  for d in range(NDT)]
            KT = [[kt_pool.tile([128, CH], BF16, tag=f'kt{c}_{j}',
                                name=f'kt{c}_{j}') for j in range(4)]
                  for c in range(NC)]
            V = [v_pool.tile([128, VW], BF16, tag=f'v{t}', name=f'v{t}')
                 for t in range(NKB)]

            # pre-warm the ACT exp table and the gpsimd library so the
            # first real exp / affine_select doesn't pay the load
            warm = sm_pool.tile([1, 16], F32, tag='warm', bufs=1)
            warm2 = sm_pool.tile([2, 16], F32, tag='warm2', bufs=1)
            nc.vector.memset(warm[:, :], 0.0)
            nc.scalar.activation(warm[:, :], warm[:, :], AF.Exp)
            nc.gpsimd.affine_select(
                out=warm[:, :], in_=warm[:, :], compare_op=ALU.is_ge,
                fill=0.0, base=0, channel_multiplier=-1, pattern=[[1, 16]])
            nc.gpsimd.partition_broadcast(warm2[:, :], warm[:, :])

            # DMAs ordered by first use: WK and x feed the first proj
            for d in range(NDT):
                nc.sync.dma_start(WK[d][:, :], wk_d[d*128:(d+1)*128, :])
                nc.sync.dma_start(XS[d][:, :], xT_d[d*128:(d+1)*128, :])
            for d in range(NDT):
                nc.sync.dma_start(WV[d][:, :], wv_d[d*128:(d+1)*128, :])
            for d in range(NDT):
                nc.sync.dma_start(WQ[d][:, :], wq_d[d*128:(d+1)*128, :])
            for d in range(4):
                nc.sync.dma_start(WO[d][:, :], wo_d[d*128:(d+1)*128, :])
            # ones columns for the softmax-denominator trick + zero pad
            for t in range(NKB):
                vt3 = V[t][:, 0:HPC*(HD+1)].rearrange(
                    'p (h c) -> p h c', c=HD + 1)
                nc.gpsimd.memset(vt3[:, :, HD], 1.0)
                nc.gpsimd.memset(V[t][:, HPC*(HD+1):VW], 0.0)

            qt_tiles = {}
            ao_tiles = {}

            def proj_group(kind, c, i):
                """One 8-matmul projection group for token chunk c."""
                tok = slice(c*CH, (c+1)*CH)
                if kind == 'K':     # KT[c][i]: out [128 dout, CH tok]
                    pp = ps_ot.tile([128, CH], F32, tag='ot', name='pp')
                    for d in range(NDT):
                        nc.tensor.matmul(
                            pp[:, :], lhsT=WK[d][:, i*128:(i+1)*128],
                            rhs=XS[d][:, tok],
                            start=(d == 0), stop=(d == NDT - 1))
                    nc.vector.tensor_copy(KT[c][i][:, :], pp[:, :])
                elif kind == 'V':   # V block c*4+i: out [128 tok, DPC dout]
                    pv = ps_ot.tile([128, DPC], F32, tag='ot', name='pv')
                    for d in range(NDT):
                        nc.tensor.matmul(
                            pv[:, :],
                            lhsT=XS[d][:, c*CH+i*128:c*CH+(i+1)*128],
                            rhs=WV[d][:, :],
                            start=(d == 0), stop=(d == NDT - 1))
                    vt3 = V[c*4 + i][:, 0:HPC*(HD+1)].rearrange(
                        'p (h c) -> p h c', c=HD + 1)
                    nc.vector.tensor_copy(
                        vt3[:, :, 0:HD],
                        pv.rearrange('p (h c) -> p h c', c=HD))
                elif kind == 'Q':   # QT[c][i]: out [128 dout, CH tok]
                    pq = ps_ot.tile([128, CH], F32, tag='ot', name='pq')
                    for d in range(NDT):
                        nc.tensor.matmul(
                            pq[:, :], lhsT=WQ[d][:, i*128:(i+1)*128],
                            rhs=XS[d][:, tok],
                            start=(d == 0), stop=(d == NDT - 1))
                    qt = qt_pool.tile([128, CH], BF16, tag=f'qt{i}',
                                      name=f'qt{c}_{i}')
                    nc.vector.tensor_copy(qt[:, :], pq[:, :])
                    qt_tiles.setdefault(c, {})[i] = qt

            def outproj_qt(c, qt_i):
                """Out-projection for query rows [c*CH + qt_i*128 ...)."""
                ao = ao_tiles[c]
                q0 = c * CH
                os = osb_pool.tile([128, D], F32, tag='os', name='os')
                for half in range(2):
                    pj = ps_ot.tile([128, 512], F32, tag='ot', name='pj')
                    for d in range(4):
                        nc.tensor.matmul(
                            pj[:, :],
                            lhsT=ao[d][:, qt_i*128:(qt_i+1)*128],
                            rhs=WO[d][:, half*512:(half+1)*512],
                            start=(d == 0), stop=(d == 3))
                    nc.vector.tensor_copy(
                        os[:, half*512:(half+1)*512], pj[:, :])
                nc.sync.dma_start(
                    po_d[q0+qt_i*128:q0+(qt_i+1)*128, :], os[:, :])

            def attention_j(c, j):
                """Causal attention for head pair j over query chunk c."""
                q0 = c * CH
                nkb = (q0 + CH) // 128
                QTj = qt_tiles[c][j]
                h0, h1 = 2*j, 2*j + 1
                ot0 = ps_ot.tile([128, CH], F32, tag='ot', name='ot0')
                ot1 = ps_ot.tile([128, CH], F32, tag='ot', name='ot1')
                pend = None
                for kbp in range(nkb // 2):
                    ka, kb = 2*kbp, 2*kbp + 1
                    lo_a = max(0, ka*128 - q0)
                    lo_b = max(0, kb*128 - q0)
                    # s_all sections: a-even | b-even | a-odd | b-odd
                    s_all = ps_s.tile([128, 4*CH], F32, tag='s', name='s')
                    pt = pt_pool.tile([128, 4*CH], BF16, tag='pt', name='pt')
                    ksa = KT[ka//4][j][:, (ka % 4)*128:((ka % 4)+1)*128]
                    ksb = KT[kb//4][j][:, (kb % 4)*128:((kb % 4)+1)*128]
                    # S, two concurrent K=64 row-tile pairs
                    nc.tensor.matmul(
                        s_all[:, lo_a:CH], lhsT=ksa[0:64, :],
                        rhs=QTj[0:64, lo_a:CH], start=True, stop=True)
                    nc.tensor.matmul(
                        s_all[:, 2*CH+lo_a:3*CH], lhsT=ksa[64:128, :],
                        rhs=QTj[64:128, lo_a:CH], start=True, stop=True)
                    nc.tensor.matmul(
                        s_all[:, CH+lo_b:2*CH], lhsT=ksb[0:64, :],
                        rhs=QTj[0:64, lo_b:CH], start=True, stop=True)
                    nc.tensor.matmul(
                        s_all[:, 3*CH+lo_b:4*CH], lhsT=ksb[64:128, :],
                        rhs=QTj[64:128, lo_b:CH], start=True, stop=True)
                    if pend is not None:
                        for (pk, pl, pc0), ppt in pend:
                            nc.tensor.matmul(
                                ot0[:, pl:CH],
                                lhsT=V[pk][:, 65*h0:65*h0+128],
                                rhs=ppt[:, pc0+pl:pc0+CH],
                                start=(pk == 0), stop=False)
                            nc.tensor.matmul(
                                ot1[:, pl:CH],
                                lhsT=V[pk][:, 65*h1:65*h1+128],
                                rhs=ppt[:, 2*CH+pc0+pl:2*CH+pc0+CH],
                                start=(pk == 0), stop=False)
                    nc.scalar.activation(
                        pt[:, lo_a:4*CH], s_all[:, lo_a:4*CH], AF.Exp)
                    for kx, lox, c0 in ((ka, lo_a, 0), (kb, lo_b, CH)):
                        if kx*128 >= q0:   # causal mask on diagonal block
                            for par in range(2):
                                nc.gpsimd.affine_select(
                                    out=pt[:, 2*CH*par+c0+lox:
                                           2*CH*par+c0+lox+128],
                                    in_=pt[:, 2*CH*par+c0+lox:
                                           2*CH*par+c0+lox+128],
                                    compare_op=ALU.is_ge, fill=0.0,
                                    base=0, channel_multiplier=-1,
                                    pattern=[[1, 128]])
                    pend = [((ka, lo_a, 0), pt), ((kb, lo_b, CH), pt)]
                for (pk, pl, pc0), ppt in pend:
                    nc.tensor.matmul(
                        ot0[:, pl:CH],
                        lhsT=V[pk][:, 65*h0:65*h0+128],
                        rhs=ppt[:, pc0+pl:pc0+CH],
                        start=(pk == 0), stop=(pk == nkb - 1))
                    nc.tensor.matmul(
                        ot1[:, pl:CH],
                        lhsT=V[pk][:, 65*h1:65*h1+128],
                        rhs=ppt[:, 2*CH+pc0+pl:2*CH+pc0+CH],
                        start=(pk == 0), stop=(pk == nkb - 1))
                # normalize both heads of the pair
                rp0 = sm_pool.tile([1, CH], F32, tag='rp0', bufs=2)
                rp1 = sm_pool.tile([1, CH], F32, tag='rp1', bufs=2)
                din0 = sm_pool.tile([1, CH], F32, tag='din0', bufs=2)
                din1 = sm_pool.tile([1, CH], F32, tag='din1', bufs=2)
                nc.vector.tensor_copy(din0[:, :], ot0[HD:HD+1, :])
                nc.vector.tensor_copy(din1[:, :], ot1[HD:HD+1, :])
                nc.vector.reciprocal_approx_fast(out=rp0[:, :], in_=din0[:, :])
                nc.vector.reciprocal_approx_fast(out=rp1[:, :], in_=din1[:, :])
                rbs0 = sm_pool.tile([HD, CH], F32, tag='rbs0', bufs=2)
                rbs1 = sm_pool.tile([HD, CH], F32, tag='rbs1', bufs=2)
                nc.gpsimd.partition_broadcast(rbs0[:, :], rp0[:, :])
                nc.gpsimd.partition_broadcast(rbs1[:, :], rp1[:, :])
                ao = ao_tiles[c][j]
                nc.vector.tensor_tensor(
                    out=ao[0:HD, :], in0=ot0[0:HD, :], in1=rbs0[:, :],
                    op=ALU.mult)
                nc.vector.tensor_tensor(
                    out=ao[HD:128, :], in0=ot1[0:HD, :], in1=rbs1[:, :],
                    op=ALU.mult)

            # ---------------- emission schedule ----------------
            for j in range(4):
                proj_group('K', 0, j)
            for tt in range(4):
                proj_group('V', 0, tt)
            for j in range(4):
                proj_group('Q', 0, j)

            # per chunk c: attention(c) woven with proj(c+1) and outproj(c-1)
            for c in range(NC):
                ao_tiles[c] = [ao_pool.tile([128, CH], BF16, tag=f'ao{j}',
                                            name=f'ao{c}_{j}')
                               for j in range(4)]
                weave = []
                if c + 1 < NC:
                    weave += [('K', c+1, i) for i in range(4)]
                    weave += [('V', c+1, i) for i in range(4)]
                    weave += [('Q', c+1, i) for i in range(4)]
                if c - 1 >= 0:
                    weave += [('O', c-1, i) for i in range(4)]
                per_j = (len(weave) + 3) // 4
                for j in range(4):
                    for kind, wc, wi in weave[j*per_j:(j+1)*per_j]:
                        if kind == 'O':
                            outproj_qt(wc, wi)
                        else:
                            proj_group(kind, wc, wi)
                    attention_j(c, j)
            # remaining out-projections (chunks 2 and 3)
            for qt_i in range(4):
                outproj_qt(2, qt_i)
            for qt_i in range(4):
                outproj_qt(3, qt_i)

    nc.compile()
    return nc


def _get_nc():
    if 'nc' not in _nc_cache:
        _nc_cache['nc'] = _build_nc()
    return _nc_cache['nc']


def kernel(x, w_qkv, w_out, _profile=False):
    import ml_dtypes
    from concourse.bass_utils import run_bass_kernel_spmd

    x = np.asarray(x, dtype=np.float32)
    w_qkv = np.asarray(w_qkv, dtype=np.float32)
    w_out = np.asarray(w_out, dtype=np.float32)

    nc = _get_nc()

    bf16 = ml_dtypes.bfloat16
    scale = np.float32(1.0 / np.sqrt(HD))
    in_maps = []
    for c in range(N_CORES):
        b, hh = c // 2, c % 2
        s, e = hh * DPC, (hh + 1) * DPC
        in_maps.append({
            'xT': np.ascontiguousarray(x[b].T).astype(bf16),
            'wq': np.ascontiguousarray(w_qkv[:, s:e] * scale).astype(bf16),
            'wk': np.ascontiguousarray(w_qkv[:, D+s:D+e]).astype(bf16),
            'wv': np.ascontiguousarray(w_qkv[:, 2*D+s:2*D+e]).astype(bf16),
            'wo': np.ascontiguousarray(w_out[s:e, :]).astype(bf16),
        })

    res = run_bass_kernel_spmd(nc, in_maps, core_ids=list(range(N_CORES)),
                               trace=_profile)
    out = np.empty((B, T, D), np.float32)
    for b in range(B):
        out[b] = res.results[2*b]['po'] + res.results[2*b+1]['po']
    if _profile:
        return out, res
    return out


# revision 7
# speedup vs baseline: 1.1019x; 1.1019x over previous
"""Causal multi-head attention on 8 TRN2 NeuronCores.

Sharding: core c -> (batch b = c // 2, head-half hh = c % 2).
Each core computes QKV for its 8 heads over the full sequence of its batch,
causal flash attention, and a partial out-projection using its 512 rows of
w_out. The host sums the two partials per batch (the "all-reduce" of the
tensor-parallel out projection).

v3 layout (all matmul operands bf16):
  xs[d]    [128, 2048] whole input row-block, loaded once
  KT[c][j] [128, 512]  K^T head pair j, token chunk c (head 2j rows 0:64,
                       head 2j+1 rows 64:128)
  V[t]     [128, 584]  V token-tile t, 8 heads x (64 cols + ones col),
                       padded so AV can load 128-col weight slices (FWL)
  QT[c][j] [128, 512]  Q^T (even head rows 0:64, odd rows 64:128)

Per k-block pair, scores for both heads land in ONE 4-bank PSUM tile
  s_all = [a-even | b-even | a-odd | b-odd]  (4 x 512 cols)
so a single wide EXP covers the pair and all four S matmuls become ready
together; emitted as (a-even,a-odd),(b-even,b-odd) they run pairwise
CONCURRENTLY on the PE via K=64 row tiling (tile_position (0,0)/(64,0)).

AV uses lhsT = V[pk][:, 65*h : 65*h+128] (128-col weight loads -> FWL);
output rows 65:127 are garbage that lands in unread PSUM partitions.

Schedule: QKV projection for chunk c+1 and the out-projection for chunk
c-1 are woven into the attention pair loop of chunk c.

Shapes (hardcoded): B=4, T=2048, D=1024, H=16, HD=64.
"""
import sys

for _p in ('/opt/trn_rl_repo', '/root/.axon_site/_ro/trn_rl_repo'):
    if _p not in sys.path:
        sys.path.insert(0, _p)

import numpy as np

B, T, D = 4, 2048, 1024
H, HD = 16, 64
HPC = H // 2          # heads per core = 8
DPC = HPC * HD        # out-dims per core = 512
N_CORES = 8

_nc_cache = {}


def _build_nc():
    import concourse.bacc as bacc
    import concourse.mybir as mybir
    from concourse.tile import TileContext

    F32 = mybir.dt.float32
    BF16 = mybir.dt.bfloat16
    AF = mybir.ActivationFunctionType
    ALU = mybir.AluOpType

    CH = 512              # token chunk (both proj and attention q-chunk)
    NKB = T // 128        # 16 k-blocks
    NC = T // CH          # 4 chunks
    NDT = D // 128        # 8 input-dim tiles
    VW = HPC * (HD + 1) + 64   # V tile width = 584 (64-col pad for LDW)

    nc = bacc.Bacc('TRN2', target_bir_lowering=False, debug=False)
    xT_d = nc.dram_tensor('xT', [D, T], BF16, kind='ExternalInput')
    wq_d = nc.dram_tensor('wq', [D, DPC], BF16, kind='ExternalInput')
    wk_d = nc.dram_tensor('wk', [D, DPC], BF16, kind='ExternalInput')
    wv_d = nc.dram_tensor('wv', [D, DPC], BF16, kind='ExternalInput')
    wo_d = nc.dram_tensor('wo', [DPC, D], BF16, kind='ExternalInput')
    po_d = nc.dram_tensor('po', [T, D], F32, kind='ExternalOutput')

    with nc.allow_low_precision(reason='bf16 matmuls by design'), \
            TileContext(nc) as tc:
        with (
            tc.tile_pool(name='w', bufs=1) as w_pool,
            tc.tile_pool(name='kt', bufs=1) as kt_pool,
            tc.tile_pool(name='vv', bufs=1) as v_pool,
            tc.tile_pool(name='xs', bufs=1) as x_pool,
            tc.tile_pool(name='qt', bufs=2) as qt_pool,
            tc.tile_pool(name='pt', bufs=3) as pt_pool,
            tc.tile_pool(name='ao', bufs=2) as ao_pool,
            tc.tile_pool(name='osb', bufs=2) as osb_pool,
            tc.tile_pool(name='small', bufs=2) as sm_pool,
            tc.tile_pool(name='ps_s', bufs=2, space='PSUM') as ps_s,
            tc.tile_pool(name='ps_ot', bufs=4, space='PSUM') as ps_ot,
        ):
            WK = [w_pool.tile([128, DPC], BF16, tag=f'wk{d}', name=f'wks{d}')
                  for d in range(NDT)]
            WV = [w_pool.tile([128, DPC], BF16, tag=f'wv{d}', name=f'wvs{d}')
                  for d in range(NDT)]
            WQ = [w_pool.tile([128, DPC], BF16, tag=f'wq{d}', name=f'wqs{d}')
                  for d in range(NDT)]
            WO = [w_pool.tile([128, D], BF16, tag=f'wo{d}', name=f'wos{d}')
                  for d in range(4)]
            XS = [x_pool.tile([128, T], BF16, tag=f'x{d}', name=f'xs{d}')
                  for d in range(NDT)]
            KT = [[kt_pool.tile([128, CH], BF16, tag=f'kt{c}_{j}',
                                name=f'kt{c}_{j}') for j in range(4)]
                  for c in range(NC)]
            V = [v_pool.tile([128, VW], BF16, tag=f'v{t}', name=f'v{t}')
                 for t in range(NKB)]

            # pre-warm the ACT exp table and the gpsimd library so the
            # first real exp / affine_select doesn't pay the load
            warm = sm_pool.tile([1, 16], F32, tag='warm', bufs=1)
            warm2 = sm_pool.tile([2, 16], F32, tag='warm2', bufs=1)
            nc.vector.memset(warm[:, :], 0.0)
            nc.scalar.activation(warm[:, :], warm[:, :], AF.Exp)
            nc.gpsimd.affine_select(
                out=warm[:, :], in_=warm[:, :], compare_op=ALU.is_ge,
                fill=0.0, base=0, channel_multiplier=-1, pattern=[[1, 16]])
            nc.gpsimd.partition_broadcast(warm2[:, :], warm[:, :])

            # DMAs ordered by first use: WK and x feed the first proj
            for d in range(NDT):
                nc.sync.dma_start(WK[d][:, :], wk_d[d*128:(d+1)*128, :])
                nc.sync.dma_start(XS[d][:, :], xT_d[d*128:(d+1)*128, :])
            for d in range(NDT):
                nc.sync.dma_start(WV[d][:, :], wv_d[d*128:(d+1)*128, :])
            for d in range(NDT):
                nc.sync.dma_start(WQ[d][:, :], wq_d[d*128:(d+1)*128, :])
            for d in range(4):
                nc.sync.dma_start(WO[d][:, :], wo_d[d*128:(d+1)*128, :])
            # ones columns for the softmax-denominator trick + zero pad
            for t in range(NKB):
                vt3 = V[t][:, 0:HPC*(HD+1)].rearrange(
                    'p (h c) -> p h c', c=HD + 1)
                nc.gpsimd.memset(vt3[:, :, HD], 1.0)
                nc.gpsimd.memset(V[t][:, HPC*(HD+1):VW], 0.0)

            qt_tiles = {}
            ao_tiles = {}

            def proj_group(kind, c, i):
                """One 8-matmul projection group for token chunk c."""
                tok = slice(c*CH, (c+1)*CH)
                if kind == 'K':     # KT[c][i]: out [128 dout, CH tok]
                    pp = ps_ot.tile([128, CH], F32, tag='ot', name='pp')
                    for d in range(NDT):
                        nc.tensor.matmul(
                            pp[:, :], lhsT=WK[d][:, i*128:(i+1)*128],
                            rhs=XS[d][:, tok],
                            start=(d == 0), stop=(d == NDT - 1))
                    nc.vector.tensor_copy(KT[c][i][:, :], pp[:, :])
                elif kind == 'V':   # V block c*4+i: out [128 tok, DPC dout]
                    pv = ps_ot.tile([128, DPC], F32, tag='ot', name='pv')
                    for d in range(NDT):
                        nc.tensor.matmul(
                            pv[:, :],
                            lhsT=XS[d][:, c*CH+i*128:c*CH+(i+1)*128],
                            rhs=WV[d][:, :],
                            start=(d == 0), stop=(d == NDT - 1))
                    vt3 = V[c*4 + i][:, 0:HPC*(HD+1)].rearrange(
                        'p (h c) -> p h c', c=HD + 1)
                    nc.vector.tensor_copy(
                        vt3[:, :, 0:HD],
                        pv.rearrange('p (h c) -> p h c', c=HD))
                elif kind == 'Q':   # QT[c][i]: out [128 dout, CH tok]
                    pq = ps_ot.tile([128, CH], F32, tag='ot', name='pq')
                    for d in range(NDT):
                        nc.tensor.matmul(
                            pq[:, :], lhsT=WQ[d][:, i*128:(i+1)*128],
                            rhs=XS[d][:, tok],
                            start=(d == 0), stop=(d == NDT - 1))
                    qt = qt_pool.tile([128, CH], BF16, tag=f'qt{i}',
                                      name=f'qt{c}_{i}')
                    nc.vector.tensor_copy(qt[:, :], pq[:, :])
                    qt_tiles.setdefault(c, {})[i] = qt

            def outproj_qt(c, qt_i):
                """Out-projection for query rows [c*CH + qt_i*128 ...)."""
                ao = ao_tiles[c]
                q0 = c * CH
                os = osb_pool.tile([128, D], F32, tag='os', name='os')
                for half in range(2):
                    pj = ps_ot.tile([128, 512], F32, tag='ot', name='pj')
                    for d in range(4):
                        nc.tensor.matmul(
                            pj[:, :],
                            lhsT=ao[d][:, qt_i*128:(qt_i+1)*128],
                            rhs=WO[d][:, half*512:(half+1)*512],
                            start=(d == 0), stop=(d == 3))
                    nc.vector.tensor_copy(
                        os[:, half*512:(half+1)*512], pj[:, :])
                nc.sync.dma_start(
                    po_d[q0+qt_i*128:q0+(qt_i+1)*128, :], os[:, :])

            def attention_j(c, j, weave_items=()):
                """Causal attention for head pair j over query chunk c.

                weave_items: filler proj/outproj groups emitted between
                k-blocks so TensorE has work during each exp shadow.
                """
                q0 = c * CH
                nkb = (q0 + CH) // 128
                QTj = qt_tiles[c][j]
                h0, h1 = 2*j, 2*j + 1
                ot0 = ps_ot.tile([128, CH], F32, tag='ot', name='ot0')
                ot1 = ps_ot.tile([128, CH], F32, tag='ot', name='ot1')
                witems = list(weave_items)
                spacing = max(1, nkb // len(witems)) if witems else 0
                pend = None
                for k in range(nkb):
                    lo = max(0, k*128 - q0)
                    # s sections: [even | odd]
                    s = ps_s.tile([128, 2*CH], F32, tag='s', name='s')
                    pt = pt_pool.tile([128, 2*CH], BF16, tag='pt', name='pt')
                    ks = KT[k//4][j][:, (k % 4)*128:((k % 4)+1)*128]
                    # S for both heads, concurrent K=64 row tiles
                    nc.tensor.matmul(
                        s[:, lo:CH], lhsT=ks[0:64, :],
                        rhs=QTj[0:64, lo:CH], start=True, stop=True)
                    nc.tensor.matmul(
                        s[:, CH+lo:2*CH], lhsT=ks[64:128, :],
                        rhs=QTj[64:128, lo:CH], start=True, stop=True)
                    if pend is not None:
                        pk, pl, ppt = pend
                        nc.tensor.matmul(
                            ot0[:, pl:CH],
                            lhsT=V[pk][:, 65*h0:65*h0+128],
                            rhs=ppt[:, pl:CH],
                            start=(pk == 0), stop=False)
                        nc.tensor.matmul(
                            ot1[:, pl:CH],
                            lhsT=V[pk][:, 65*h1:65*h1+128],
                            rhs=ppt[:, CH+pl:2*CH],
                            start=(pk == 0), stop=False)
                    nc.scalar.activation(
                        pt[:, lo:2*CH], s[:, lo:2*CH], AF.Exp)
                    if k*128 >= q0:   # causal mask on diagonal block
                        for c0 in (0, CH):
                            nc.gpsimd.affine_select(
                                out=pt[:, c0+lo:c0+lo+128],
                                in_=pt[:, c0+lo:c0+lo+128],
                                compare_op=ALU.is_ge, fill=0.0,
                                base=0, channel_multiplier=-1,
                                pattern=[[1, 128]])
                    if witems and k % spacing == spacing - 1:
                        kind, wc, wi = witems.pop(0)
                        if kind == 'O':
                            outproj_qt(wc, wi)
                        else:
                            proj_group(kind, wc, wi)
                    pend = (k, lo, pt)
                pk, pl, ppt = pend
                nc.tensor.matmul(
                    ot0[:, pl:CH],
                    lhsT=V[pk][:, 65*h0:65*h0+128],
                    rhs=ppt[:, pl:CH],
                    start=(pk == 0), stop=(pk == nkb - 1))
                nc.tensor.matmul(
                    ot1[:, pl:CH],
                    lhsT=V[pk][:, 65*h1:65*h1+128],
                    rhs=ppt[:, CH+pl:2*CH],
                    start=(pk == 0), stop=(pk == nkb - 1))
                for kind, wc, wi in witems:   # leftovers
                    if kind == 'O':
                        outproj_qt(wc, wi)
                    else:
                        proj_group(kind, wc, wi)
                # normalize both heads of the pair
                rp0 = sm_pool.tile([1, CH], F32, tag='rp0', bufs=2)
                rp1 = sm_pool.tile([1, CH], F32, tag='rp1', bufs=2)
                din0 = sm_pool.tile([1, CH], F32, tag='din0', bufs=2)
                din1 = sm_pool.tile([1, CH], F32, tag='din1', bufs=2)
                nc.vector.tensor_copy(din0[:, :], ot0[HD:HD+1, :])
                nc.vector.tensor_copy(din1[:, :], ot1[HD:HD+1, :])
                nc.vector.reciprocal_approx_fast(out=rp0[:, :], in_=din0[:, :])
                nc.vector.reciprocal_approx_fast(out=rp1[:, :], in_=din1[:, :])
                rbs0 = sm_pool.tile([HD, CH], F32, tag='rbs0', bufs=2)
                rbs1 = sm_pool.tile([HD, CH], F32, tag='rbs1', bufs=2)
                nc.gpsimd.partition_broadcast(rbs0[:, :], rp0[:, :])
                nc.gpsimd.partition_broadcast(rbs1[:, :], rp1[:, :])
                ao = ao_tiles[c][j]
                nc.vector.tensor_tensor(
                    out=ao[0:HD, :], in0=ot0[0:HD, :], in1=rbs0[:, :],
                    op=ALU.mult)
                nc.vector.tensor_tensor(
                    out=ao[HD:128, :], in0=ot1[0:HD, :], in1=rbs1[:, :],
                    op=ALU.mult)

            # ---------------- emission schedule ----------------
            for j in range(4):
                proj_group('K', 0, j)
            for tt in range(4):
                proj_group('V', 0, tt)
            for j in range(4):
                proj_group('Q', 0, j)

            # per chunk c: attention(c) woven with proj(c+1) and outproj(c-1)
            for c in range(NC):
                ao_tiles[c] = [ao_pool.tile([128, CH], BF16, tag=f'ao{j}',
                                            name=f'ao{c}_{j}')
                               for j in range(4)]
                weave = []
                if c + 1 < NC:
                    weave += [('K', c+1, i) for i in range(4)]
                    weave += [('V', c+1, i) for i in range(4)]
                    weave += [('Q', c+1, i) for i in range(4)]
                if c - 1 >= 0:
                    weave += [('O', c-1, i) for i in range(4)]
                per_j = (len(weave) + 3) // 4
                for j in range(4):
                    attention_j(c, j, weave[j*per_j:(j+1)*per_j])
            # chunk 3's out-projection: inherently the tail
            for qt_i in range(4):
                outproj_qt(3, qt_i)

    nc.compile()
    return nc


def _get_nc():
    if 'nc' not in _nc_cache:
        _nc_cache['nc'] = _build_nc()
    return _nc_cache['nc']


def kernel(x, w_qkv, w_out, _profile=False):
    import ml_dtypes
    from concourse.bass_utils import run_bass_kernel_spmd

    x = np.asarray(x, dtype=np.float32)
    w_qkv = np.asarray(w_qkv, dtype=np.float32)
    w_out = np.asarray(w_out, dtype=np.float32)

    nc = _get_nc()

    bf16 = ml_dtypes.bfloat16
    scale = np.float32(1.0 / np.sqrt(HD))
    in_maps = []
    for c in range(N_CORES):
        b, hh = c // 2, c % 2
        s, e = hh * DPC, (hh + 1) * DPC
        in_maps.append({
            'xT': np.ascontiguousarray(x[b].T).astype(bf16),
            'wq': np.ascontiguousarray(w_qkv[:, s:e] * scale).astype(bf16),
            'wk': np.ascontiguousarray(w_qkv[:, D+s:D+e]).astype(bf16),
            'wv': np.ascontiguousarray(w_qkv[:, 2*D+s:2*D+e]).astype(bf16),
            'wo': np.ascontiguousarray(w_out[s:e, :]).astype(bf16),
        })

    res = run_bass_kernel_spmd(nc, in_maps, core_ids=list(range(N_CORES)),
                               trace=_profile)
    out = np.empty((B, T, D), np.float32)
    for b in range(B):
        out[b] = res.results[2*b]['po'] + res.results[2*b+1]['po']
    if _profile:
        return out, res
    return out
